# revision 7
# baseline (speedup 1.0000x reference)
"""Embedding lookup kernel for Trainium2 (8 NeuronCores, SPMD).

Strategy: token-parallel gather (an embedding lookup IS a row gather:
out[b, s, :] = weight[x[b, s], :]).

- Flatten x [2, 4096] -> [8192] tokens; each of the 8 cores handles 1024
  contiguous tokens. Each core gets the FULL weight table in its DRAM.
- Per core (raw Bacc program, no Tile framework overhead; the Bass entry
  all-engine barrier is skipped and the unused PE/DVE/ACT engine
  preambles + const-AP memsets are stripped — every cross-engine
  dependency below is ordered by an explicit semaphore):
    1. Pool issues the GPSIMD `mlp` library load FIRST (async ~8.8us on
       the Q7 cores: the real critical path of this kernel) while, in
       parallel, one HWDGE DMA loads the 1024 indices as int16 [128, 64]
       into SBUF (index t at partition t%16, column t//16, replicated
       across the 8 16-partition groups — each Q7 core reads only its
       own group).
    2. Four dma_gather instructions, one per SWDGE queue 0-3 (each queue
       = its own Q7 core pair), each gather 256 rows (512B each) using
       idx columns [16q, 16q+16). Descriptor generation (~994ns +
       7.5ns/row) runs CONCURRENTLY on the 4 pairs instead of 8 serial
       indirect-DMA launches at ~1.4us each on pair 0. Gather q lands in
       SBUF g[128, 2q+b, 128]: token 256q+128b+p at partition p.
    3. As each gather's completion semaphore fires, an HWDGE DMA writes
       its two 128-row blocks to DRAM out[:, 2q:2q+2, :], overlapping
       the remaining gathers' drains. No final completion wait: the NEFF
       epilogue's engine drains already block until the queues are empty.
- Host: out.reshape(128, 8, 128).transpose(1, 0, 2) -> [1024, 128] rows
  in token order (row j*128+p = token j*128+p); concatenate the 8
  per-core outputs.

No collectives. Bit-exact vs the one-hot matmul reference.
"""

import contextlib

import numpy as np

import concourse.bass as bass
from concourse import bacc, mybir
from concourse.bass_utils import run_bass_kernel_spmd
from concourse.library_config import mlp

N_CORES = 8
B, S = 2, 4096
VOCAB, DIM = 32000, 128
P = 128
TOKENS = B * S                      # 8192
TPC = TOKENS // N_CORES             # 1024 tokens per core
IDX_COLS = TPC // 16                # 64 int16 index columns
BLOCKS = TPC // P                   # 8 gathered row-blocks per partition
NQ = 4                              # SWDGE queues (= Q7 core pairs) used
IPQ = TPC // NQ                     # 256 indices per queue


def build_nc():
    # Skip the Bass-constructor entry barrier (gates the first DMA behind
    # all engines' init); restore the method right after construction.
    orig_barrier = bass.Bass.all_engine_barrier
    bass.Bass.all_engine_barrier = lambda self, *a, **k: None
    try:
        nc = bacc.Bacc(None, target_bir_lowering=False, num_swdge_queues=NQ)
    finally:
        bass.Bass.all_engine_barrier = orig_barrier

    # Strip the construction-time preamble of engines this kernel never
    # uses (PE/DVE/ACT reg-moves + TPB-base loads) and the const-AP
    # memsets — nothing below references them.
    strip = {mybir.EngineType.PE, mybir.EngineType.DVE, mybir.EngineType.Activation}
    blk = nc.main_func.blocks[0]
    blk.instructions[:] = [
        inst
        for inst in blk.instructions
        if getattr(inst, "engine", None) not in strip
        and not isinstance(inst, mybir.InstMemset)
    ]

    x = nc.dram_tensor("x", [P, IDX_COLS], mybir.dt.int16, kind="ExternalInput")
    w = nc.dram_tensor("weight", [VOCAB, DIM], mybir.dt.float32, kind="ExternalInput")
    out = nc.dram_tensor("out", [P, BLOCKS, DIM], mybir.dt.float32, kind="ExternalOutput")

    with contextlib.ExitStack() as ctx:
        idx_tile = ctx.enter_context(
            nc.sbuf_tensor("idx_tile", [P, IDX_COLS], mybir.dt.int16)
        )
        g = ctx.enter_context(nc.sbuf_tensor("g", [P, BLOCKS, DIM], mybir.dt.float32))
        s_idx = ctx.enter_context(nc.semaphore("s_idx"))
        s_out = ctx.enter_context(nc.semaphore("s_out"))
        s_gs = [ctx.enter_context(nc.semaphore(f"s_g{q}")) for q in range(NQ)]

        # Async Q7 library load first — overlaps the idx DMA and most of
        # the NEFF prologue tail.
        nc.gpsimd.load_library(mlp)

        nc.sync.dma_start(idx_tile[:], x[:]).then_inc(s_idx, 16)

        nc.gpsimd.wait_ge(s_idx, 16)
        for q in range(NQ):
            nc.gpsimd.dma_gather(
                g[:, 2 * q : 2 * q + 2, :],
                w[:],
                idx_tile[:, 16 * q : 16 * q + 16],
                IPQ,
                IPQ,
                DIM,
                queue_num=q,
            ).then_inc(s_gs[q], 16)

        for q in range(NQ):
            nc.sync.wait_ge(s_gs[q], 16)
            nc.sync.dma_start(
                out[:, 2 * q : 2 * q + 2, :], g[:, 2 * q : 2 * q + 2, :]
            ).then_inc(s_out, 16)
    nc.compile()
    return nc


_NC_CACHE = None


def kernel(x: np.ndarray, weight: np.ndarray, **run_kwargs):
    global _NC_CACHE
    if _NC_CACHE is None:
        _NC_CACHE = build_nc()
    nc = _NC_CACHE

    x_flat = np.asarray(x).reshape(-1).astype(np.int16)
    w = np.ascontiguousarray(np.asarray(weight, dtype=np.float32))

    in_maps = []
    for c in range(N_CORES):
        tc = x_flat[c * TPC : (c + 1) * TPC]
        # index t at (partition t%16, column t//16), replicated to all 8
        # 16-partition groups
        arr16 = np.ascontiguousarray(np.tile(tc.reshape(IDX_COLS, 16).T, (8, 1)))
        in_maps.append({"x": arr16, "weight": w})
    res = run_bass_kernel_spmd(nc, in_maps, core_ids=list(range(N_CORES)), **run_kwargs)
    # out [128, 8, 128]: token j*128+p at [p, j, :]
    parts = [
        res.results[c]["out"].reshape(P, BLOCKS, DIM).transpose(1, 0, 2).reshape(TPC, DIM)
        for c in range(N_CORES)
    ]
    full = np.concatenate(parts, axis=0).reshape(B, S, DIM)
    if run_kwargs:
        return full, res
    return full
